# revision 1
# baseline (speedup 1.0000x reference)
"""Trainium2 Bass kernel for nn_MultiHeadAttention_824633721543.

MHA with periodic prefix mask: allowed iff (q % 256) >= (k % 256).
B=2, S=2048, D=768, H=12, Dk=64, WINDOW=256.

Sharding: 8 cores = 2 batches x 4 head-groups (3 heads each). Each core
computes q/k/v projections for its heads, the masked softmax attention, and
a partial O-projection; the host sums the 4 partials per batch and adds bo.

Device-side layout choices (all transpose-free):
  - scores computed as S^T [k,q]: kT slice stationary, qT moving
  - q columns tile-permuted (even 128-tiles | odd 128-tiles) so the mask is:
      even-group x k-lo  -> one shared 128x128 triu tile (0/1 multiply)
      odd-group  x k-lo  -> unmasked
      odd-group  x k-hi  -> shared triu
      even-group x k-hi  -> fully masked, never computed
  - exp on ACT (scale=1/8 folded in), no max-subtraction (scores are small)
  - P@V with [V|1] stationary -> out^T plus denominator row, accumulated over
    the 8 windows in PSUM; normalization via K=1 broadcast matmul + DVE
  - O-projection consumes attn^T directly as the stationary operand
All matmuls run in float32r (fp32 bits, 1 cycle/row at N>=256).
"""

import sys

sys.path.insert(0, "/opt/trn_rl_repo")

import numpy as np

B = 2
S = 2048
D = 768
DK = 64
WIN = 256
NW = S // WIN   # 8 windows
NHC = 3         # heads per core
DH = NHC * DK   # 192
NT = S // 128   # 16 q tiles

_CACHE = {}


def _build_program():
    import concourse.tile as tile
    from concourse import mybir, bacc
    from contextlib import ExitStack

    f32 = mybir.dt.float32
    f32r = mybir.dt.float32r
    Exp = mybir.ActivationFunctionType.Exp
    mult = mybir.AluOpType.mult
    add = mybir.AluOpType.add

    nc = bacc.Bacc("TRN2", target_bir_lowering=False, debug=False)

    xT = nc.dram_tensor("xT", [D, S], f32r, kind="ExternalInput").ap()
    w1 = nc.dram_tensor("w1", [D, 256], f32r, kind="ExternalInput").ap()  # [qh0|qh1|kh0|kh1]
    w2 = nc.dram_tensor("w2", [D, 128], f32r, kind="ExternalInput").ap()  # [qh2|kh2]
    wv = nc.dram_tensor("wv", [D, 256], f32r, kind="ExternalInput").ap()  # WvT pad
    wo = nc.dram_tensor("wo", [DH, D], f32r, kind="ExternalInput").ap()
    btA = nc.dram_tensor("btA", [128, 1], f32, kind="ExternalInput").ap()
    btB = nc.dram_tensor("btB", [128, 1], f32, kind="ExternalInput").ap()
    btC = nc.dram_tensor("btC", [64, 1], f32, kind="ExternalInput").ap()
    btD = nc.dram_tensor("btD", [64, 1], f32, kind="ExternalInput").ap()
    bvb = nc.dram_tensor("bvb", [128, 192], f32, kind="ExternalInput").ap()
    triu = nc.dram_tensor("triu", [128, 128], f32, kind="ExternalInput").ap()
    onesd = nc.dram_tensor("onesd", [128, 64], f32r, kind="ExternalInput").ap()
    out = nc.dram_tensor("out", [S, D], f32, kind="ExternalOutput").ap()

    with tile.TileContext(nc) as tc, ExitStack() as ctx:
        consts = ctx.enter_context(tc.tile_pool(name="consts", bufs=1))
        qkv = ctx.enter_context(tc.tile_pool(name="qkv", bufs=1))

        xtp_cm = tc.tile_pool(name="xtp", bufs=1)
        xtp = xtp_cm.__enter__()
        xT_sb = [xtp.tile([128, S], f32r, tag=f"xt{k}", name=f"xt{k}")
                 for k in range(6)]
        w1_sb = [consts.tile([128, 256], f32r, tag=f"w1_{k}", name=f"w1s{k}")
                 for k in range(6)]
        w2_sb = [consts.tile([128, 128], f32r, tag=f"w2_{k}", name=f"w2s{k}")
                 for k in range(6)]
        wv_sb = [consts.tile([128, 256], f32r, tag=f"wv_{k}", name=f"wvs{k}")
                 for k in range(6)]
        for k in range(6):
            nc.sync.dma_start(out=xT_sb[k], in_=xT[k * 128:(k + 1) * 128, :])
            nc.sync.dma_start(out=w1_sb[k], in_=w1[k * 128:(k + 1) * 128, :])
            nc.sync.dma_start(out=w2_sb[k], in_=w2[k * 128:(k + 1) * 128, :])
            nc.sync.dma_start(out=wv_sb[k], in_=wv[k * 128:(k + 1) * 128, :])
        # Wo^T slice split per head: three [64, 768] tiles (partition base 0)
        wo_sb = [consts.tile([64, D], f32r, tag=f"wo{h}", name=f"wos{h}")
                 for h in range(NHC)]
        for h in range(NHC):
            nc.sync.dma_start(out=wo_sb[h], in_=wo[64 * h:64 * (h + 1), :])
        btA_sb = consts.tile([128, 1], f32, tag="btA")
        btB_sb = consts.tile([128, 1], f32, tag="btB")
        btC_sb = consts.tile([64, 1], f32, tag="btC")
        btD_sb = consts.tile([64, 1], f32, tag="btD")
        nc.sync.dma_start(out=btA_sb, in_=btA)
        nc.sync.dma_start(out=btB_sb, in_=btB)
        nc.sync.dma_start(out=btC_sb, in_=btC)
        nc.sync.dma_start(out=btD_sb, in_=btD)
        bvb_sb = consts.tile([128, 192], f32, tag="bvb")
        nc.sync.dma_start(out=bvb_sb, in_=bvb)
        triu_sb = consts.tile([128, 128], f32, tag="triu")
        nc.sync.dma_start(out=triu_sb, in_=triu)
        ones_row = consts.tile([128, 64], f32r, tag="ones_row")
        nc.sync.dma_start(out=ones_row, in_=onesd)

        # ---- long-lived activation tiles ----
        tileA = qkv.tile([128, S], f32r, tag="tileA")  # [qT_h0|qT_h1], q-permuted
        tileB = qkv.tile([128, S], f32r, tag="tileB")  # [kT_h0|kT_h1], natural
        tileC = qkv.tile([64, S], f32r, tag="tileC")   # qT_h2, permuted
        tileD = qkv.tile([64, S], f32r, tag="tileD")   # kT_h2, natural
        # v natural [s,d] per s-tile: three 65-col groups [V_h | 1]
        v_sb = [qkv.tile([128, 196], f32r, tag=f"v{i}", name=f"vsb{i}")
                for i in range(NT)]
        # attn^T per head, partition base 0
        attnT = [qkv.tile([64, S], f32r, tag=f"attnT{h}", name=f"attnT{h}")
                 for h in range(NHC)]

        def permuted_copy(dst, rows, ps, n, bias):
            """psum 512-span n -> dst cols with even/odd tile permutation."""
            pr3 = ps[0:rows, :].rearrange("p (c two k) -> p c two k", two=2, k=128)
            dr = dst[0:rows, :]
            nc.vector.tensor_scalar_add(
                out=dr[:, 256 * n:256 * n + 256].rearrange("p (c k) -> p c k", k=128),
                in0=pr3[:, :, 0, :], scalar1=bias[0:rows, :])
            nc.vector.tensor_scalar_add(
                out=dr[:, 1024 + 256 * n:1024 + 256 * n + 256].rearrange(
                    "p (c k) -> p c k", k=128),
                in0=pr3[:, :, 1, :], scalar1=bias[0:rows, :])

        # ---- stage A ----
        with tc.tile_pool(name="psA", bufs=2, space="PSUM") as psA:
            for n in range(4):
                xn = [xT_sb[k][:, 512 * n:512 * (n + 1)]
                      for k in range(6)]
                psa = psA.tile([128, 512], f32, tag="psA")
                for k in range(6):
                    nc.tensor.matmul(psa, w1_sb[k][:, 0:128], xn[k],
                                     start=(k == 0), stop=(k == 5))
                permuted_copy(tileA, 128, psa, n, btA_sb)
                psb = psA.tile([128, 512], f32, tag="psA")
                for k in range(6):
                    nc.tensor.matmul(psb, w1_sb[k][:, 128:256], xn[k],
                                     start=(k == 0), stop=(k == 5))
                nc.vector.tensor_scalar_add(
                    out=tileB[:, 512 * n:512 * (n + 1)], in0=psb, scalar1=btB_sb)
                psq = psA.tile([64, 512], f32, tag="psq")
                psk = psA.tile([64, 512], f32, tag="psq")
                for k in range(6):
                    nc.tensor.matmul(psq, w2_sb[k][:, 0:64], xn[k],
                                     start=(k == 0), stop=(k == 5))
                    nc.tensor.matmul(psk, w2_sb[k][:, 64:128], xn[k],
                                     start=(k == 0), stop=(k == 5))
                permuted_copy(tileC, 64, psq, n, btC_sb)
                nc.vector.tensor_scalar_add(
                    out=tileD[:, 512 * n:512 * (n + 1)], in0=psk, scalar1=btD_sb)

            for st in range(NT):
                psv = psA.tile([128, 256], f32, tag="psv")
                for k in range(6):
                    nc.tensor.matmul(
                        psv, xT_sb[k][:, 128 * st:128 * (st + 1)],
                        wv_sb[k], start=(k == 0), stop=(k == 5))
                vt = v_sb[st]
                # copy the 3 heads' 64-col blocks into 65-col groups + bias
                nc.vector.tensor_tensor(
                    out=vt[:, 0:195].rearrange("p (h c) -> p h c", c=65)[:, :, 0:64],
                    in0=psv[:, 0:192].rearrange("p (h c) -> p h c", c=64),
                    in1=bvb_sb.rearrange("p (h c) -> p h c", c=64), op=add)
                # ones columns at 64, 129, 194
                nc.vector.tensor_copy(
                    out=vt[:, 0:195].rearrange("p (h c) -> p h c", c=65)[:, :, 64:65],
                    in_=ones_row[:, 0:3].unsqueeze(2))

        xtp_cm.__exit__(None, None, None)

        # ---- stage B ----
        heads = [
            dict(q=(tileA, 0), k=(tileB, 0)),
            dict(q=(tileA, 64), k=(tileB, 64)),
            dict(q=(tileC, 0), k=(tileD, 0)),
        ]
        triu_b = triu_sb.unsqueeze(1).broadcast_to([128, 8, 128])

        with tc.tile_pool(name="pt", bufs=6) as pt_pool, \
             tc.tile_pool(name="sc", bufs=2, space="PSUM") as sc_pool, \
             tc.tile_pool(name="po", bufs=2, space="PSUM") as out_pool, \
             tc.tile_pool(name="nrm", bufs=2) as nrm_pool:
            for h in range(NHC):
                hd = heads[h]
                qt, qoff = hd["q"]
                kt, koff = hd["k"]
                qv = qt[qoff:qoff + 64, :]
                kv = kt[koff:koff + 64, :]

                for grp in range(2):  # 0=even q-tiles, 1=odd
                    qcols = qv[:, 1024 * grp:1024 * (grp + 1)]
                    po = out_pool.tile([128, 1024], f32, tag="po")
                    state = {"first": [True, True]}

                    def pv_mm(vtile, pt, last):
                        vsl = vtile[:, 65 * h:65 * h + 65]  # [V_h | 1]
                        for sub in range(2):
                            nc.tensor.matmul(
                                po[0:65, 512 * sub:512 * (sub + 1)],
                                vsl,
                                pt[:, 512 * sub:512 * (sub + 1)],
                                start=state["first"][sub], stop=last)
                            state["first"][sub] = False

                    def scores_exp(kblk, mask, mask_eng="dve"):
                        sc = sc_pool.tile([128, 1024], f32, tag="sc")
                        for sub in range(2):
                            nc.tensor.matmul(
                                sc[:, 512 * sub:512 * (sub + 1)], kblk,
                                qcols[:, 512 * sub:512 * (sub + 1)],
                                start=True, stop=True)
                        pt = pt_pool.tile([128, 1024], f32r, tag="pt")
                        nc.scalar.activation(out=pt, in_=sc, func=Exp, scale=0.125)
                        if mask:
                            p3 = pt.rearrange("p (c k) -> p c k", k=128)
                            eng = nc.vector if mask_eng == "dve" else nc.gpsimd
                            eng.tensor_mul(out=p3, in0=p3, in1=triu_b)
                        return pt

                    for w in range(NW):
                        klo = kv[:, WIN * w:WIN * w + 128]
                        if grp == 0:
                            pt = scores_exp(klo, mask=True)
                            pv_mm(v_sb[2 * w], pt, last=(w == NW - 1))
                        else:
                            khi = kv[:, WIN * w + 128:WIN * w + 256]
                            ptlo = scores_exp(klo, mask=False)
                            pthi = scores_exp(khi, mask=True, mask_eng="gpsimd")
                            pv_mm(v_sb[2 * w], ptlo, last=False)
                            pv_mm(v_sb[2 * w + 1], pthi, last=(w == NW - 1))

                    # normalization: denom row 64 -> bcast -> recip -> mul
                    den_sb = nrm_pool.tile([128, 1024], f32r, tag="den")
                    nc.vector.tensor_copy(out=den_sb[64:65, :], in_=po[64:65, :])
                    rec_ps = sc_pool.tile([128, 1024], f32, tag="sc")
                    for sub in range(2):
                        nc.tensor.matmul(
                            rec_ps[0:64, 512 * sub:512 * (sub + 1)],
                            ones_row[64:65, :],
                            den_sb[64:65, 512 * sub:512 * (sub + 1)],
                            start=True, stop=True)
                    rec_sb = nrm_pool.tile([128, 1024], f32, tag="rec")
                    nc.vector.reciprocal_approx_fast(
                        out=rec_sb[0:64, :], in_=rec_ps[0:64, :])
                    nc.vector.tensor_tensor(
                        out=attnT[h][:, 1024 * grp:1024 * (grp + 1)],
                        in0=po[0:64, :], in1=rec_sb[0:64, :], op=mult)

        # ---- stage C ----
        with tc.tile_pool(name="oc", bufs=3, space="PSUM") as oc_pool, \
             tc.tile_pool(name="ost", bufs=3) as ost_pool:
            for p in range(NT):
                pso = oc_pool.tile([128, D], f32, tag="pso")
                for (n0, n1) in ((0, 512), (512, 768)):
                    for h in range(NHC):
                        nc.tensor.matmul(
                            pso[:, n0:n1],
                            attnT[h][:, 128 * p:128 * (p + 1)],
                            wo_sb[h][:, n0:n1],
                            start=(h == 0), stop=(h == NHC - 1))
                ot = ost_pool.tile([128, D], f32, tag="ot")
                nc.scalar.copy(out=ot, in_=pso)
                t = 2 * p if p < 8 else 2 * (p - 8) + 1
                nc.sync.dma_start(out=out[128 * t:128 * (t + 1), :], in_=ot)

    nc.compile()
    return nc


def _prep_core_inputs(inputs, c):
    x = inputs["x"]
    Wq, bq = inputs["Wq"], inputs["bq"]
    Wk, bk = inputs["Wk"], inputs["bk"]
    Wv, bv = inputs["Wv"], inputs["bv"]
    Wo = inputs["Wo"]
    b = c // 4
    r0 = (c % 4) * DH  # first feature row of this core's 192-row head block

    xT = np.ascontiguousarray(np.asarray(x[b]).T.astype(np.float32))
    W1 = np.ascontiguousarray(np.concatenate(
        [Wq[r0:r0 + 128].T, Wk[r0:r0 + 128].T], axis=1).astype(np.float32))
    W2 = np.ascontiguousarray(np.concatenate(
        [Wq[r0 + 128:r0 + 192].T, Wk[r0 + 128:r0 + 192].T], axis=1).astype(np.float32))
    Wvp = np.zeros((D, 256), np.float32)
    Wvp[:, 0:192] = Wv[r0:r0 + 192].T
    wo = np.ascontiguousarray(Wo[:, r0:r0 + 192].T.astype(np.float32))

    return dict(
        xT=xT, w1=W1, w2=W2, wv=Wvp, wo=wo,
        btA=np.ascontiguousarray(bq[r0:r0 + 128].reshape(128, 1).astype(np.float32)),
        btB=np.ascontiguousarray(bk[r0:r0 + 128].reshape(128, 1).astype(np.float32)),
        btC=np.ascontiguousarray(bq[r0 + 128:r0 + 192].reshape(64, 1).astype(np.float32)),
        btD=np.ascontiguousarray(bk[r0 + 128:r0 + 192].reshape(64, 1).astype(np.float32)),
        bvb=np.ascontiguousarray(np.tile(
            bv[r0:r0 + 192].reshape(1, 192), (128, 1)).astype(np.float32)),
        triu=np.ascontiguousarray(np.triu(np.ones((128, 128), np.float32))),
        onesd=np.ones((128, 64), np.float32),
    )


def _install_ntff_hook():
    """Register antenv.axon_hooks with a ctypes NTFF profile hook so
    run_bass_kernel_spmd(trace=True) can capture device-side exec time."""
    import types, ctypes, contextlib, importlib

    try:
        import antenv.axon_hooks  # noqa: F401
        return
    except ImportError:
        pass
    so_path = "/opt/axon/libaxon_pjrt.so"
    lib = ctypes.CDLL(so_path)
    if not hasattr(lib, "axon_start_nrt_profile"):
        return
    lib.axon_start_nrt_profile.argtypes = [
        ctypes.POINTER(ctypes.c_int64), ctypes.c_size_t]
    lib.axon_start_nrt_profile.restype = ctypes.c_int64
    lib.axon_stop_nrt_profile.argtypes = [ctypes.c_char_p]
    lib.axon_stop_nrt_profile.restype = ctypes.c_int64

    @contextlib.contextmanager
    def _hook(output_dir, device_ids):
        import jax
        jax.devices()
        if device_ids:
            ids = (ctypes.c_int64 * len(device_ids))(*device_ids)
            rc = lib.axon_start_nrt_profile(ids, len(device_ids))
        else:
            rc = lib.axon_start_nrt_profile(None, 0)
        if rc != 0:
            raise RuntimeError(f"axon_start_nrt_profile rc={rc}")
        try:
            yield
        finally:
            n = lib.axon_stop_nrt_profile(str(output_dir).encode())
            print(f"profile: {n} file(s) written to {output_dir}")

    mod = types.ModuleType("antenv.axon_hooks")
    mod.get_axon_ntff_profile_hook = lambda: _hook
    mod.set_axon_ntff_profile_hook = lambda h: None
    sys.modules["antenv.axon_hooks"] = mod
    import antenv
    antenv.axon_hooks = mod


def kernel(**inputs):
    import os
    from concourse import bass_utils

    if "nc" not in _CACHE:
        _CACHE["nc"] = _build_program()
    nc = _CACHE["nc"]

    trace = bool(os.environ.get("MHA_TRACE"))
    kwargs = {}
    if trace:
        _install_ntff_hook()
        kwargs = dict(trace=True, tmpdir="/tmp/mha_trace")
        os.makedirs("/tmp/mha_trace", exist_ok=True)

    in_maps = [_prep_core_inputs(inputs, c) for c in range(8)]
    res = bass_utils.run_bass_kernel_spmd(
        nc, in_maps, core_ids=list(range(8)), **kwargs)
    _CACHE["last_results"] = res
    if trace and res.exec_time_ns is not None:
        print(f"HW exec time: {res.exec_time_ns} ns")
    out = np.zeros((B, S, D), np.float32)
    for c in range(8):
        out[c // 4] += res.results[c]["out"]
    out += np.asarray(inputs["bo"], np.float32).reshape(1, 1, D)
    return out



# revision 12
# speedup vs baseline: 1.1873x; 1.1873x over previous
"""Trainium2 Bass kernel for nn_MultiHeadAttention_824633721543.

MHA with periodic prefix mask: allowed iff (q % 256) >= (k % 256).
B=2, S=2048, D=768, H=12, Dk=64, WINDOW=256.

Sharding: 8 cores = 2 batches x 4 head-groups (3 heads each). Each core
computes q/k/v projections for its heads, the masked softmax attention, and
a partial O-projection; the host sums the 4 partials per batch and adds bo.

v2 design (all activations/weights bf16, PSUM accumulation fp32):
  - scores computed as S^T [k,q]; q columns tile-permuted (even 128-tiles |
    odd 128-tiles) so masks reduce to one shared 128x128 triu tile.
  - h0/h1 q,k stacked on partition halves of one [128,S] tile; score matmuls
    for both heads issued as K=64 row-tiled pairs (tile_position (0,0)/(64,0))
    that run concurrently in the PE array.
  - h2 q,k duplicated onto both partition halves (free: the projection
    matmul's stationary has spare M), so h2's two q-groups pack the same way.
  - exp on ACT (scale=1/8), bf16 out; mask = DVE bf16 multiply (4x mode).
  - P@V with [V|1] stationary -> out^T plus denominator row, accumulated over
    windows in PSUM; normalization via K=1 broadcast matmul + DVE.
  - stage A for h2 is emitted AFTER stage B of h0/h1 so the Tile scheduler
    uses it as PE filler while ACT chews exp (keeps the PE HAM-warm).
  - O-projection: h0/h1 as one K=128 matmul, h2 K=64; output DMA'd as bf16.
"""

import sys

sys.path.insert(0, "/opt/trn_rl_repo")

import numpy as np
import ml_dtypes

BF16 = ml_dtypes.bfloat16

B = 2
S = 2048
D = 768
DK = 64
WIN = 256
NW = S // WIN   # 8 windows
NHC = 3         # heads per core
DH = NHC * DK   # 192
NT = S // 128   # 16 q tiles

_CACHE = {}


def _build_program():
    import concourse.tile as tile
    from concourse import mybir, bacc
    from contextlib import ExitStack

    f32 = mybir.dt.float32
    f32r = mybir.dt.float32r
    bf16 = mybir.dt.bfloat16
    Exp = mybir.ActivationFunctionType.Exp
    mult = mybir.AluOpType.mult

    nc = bacc.Bacc("TRN2", target_bir_lowering=False, debug=False)

    xT = nc.dram_tensor("xT", [D, S], bf16, kind="ExternalInput").ap()
    w1 = nc.dram_tensor("w1", [D, 256], bf16, kind="ExternalInput").ap()  # [q01|k01]
    w2 = nc.dram_tensor("w2", [D, 256], bf16, kind="ExternalInput").ap()  # [q2 dup|k2 dup]
    wv = nc.dram_tensor("wv", [D, 192], bf16, kind="ExternalInput").ap()
    wo1 = nc.dram_tensor("wo1", [128, D], bf16, kind="ExternalInput").ap()
    wo2 = nc.dram_tensor("wo2", [64, D], bf16, kind="ExternalInput").ap()
    btA = nc.dram_tensor("btA", [128, 1], f32, kind="ExternalInput").ap()
    btB = nc.dram_tensor("btB", [128, 1], f32, kind="ExternalInput").ap()
    btC = nc.dram_tensor("btC", [128, 1], f32, kind="ExternalInput").ap()
    btD = nc.dram_tensor("btD", [128, 1], f32, kind="ExternalInput").ap()
    bvb = nc.dram_tensor("bvb", [128, 192], f32, kind="ExternalInput").ap()
    triu = nc.dram_tensor("triu", [128, 128], bf16, kind="ExternalInput").ap()
    onesb = nc.dram_tensor("onesb", [128, 64], bf16, kind="ExternalInput").ap()
    onesr = nc.dram_tensor("onesr", [1, 64], f32r, kind="ExternalInput").ap()
    out = nc.dram_tensor("out", [S, D], bf16, kind="ExternalOutput").ap()

    with tile.TileContext(nc) as tc, ExitStack() as ctx:
        consts = ctx.enter_context(tc.tile_pool(name="consts", bufs=1))
        qkv = ctx.enter_context(tc.tile_pool(name="qkv", bufs=1))

        xtp = ctx.enter_context(tc.tile_pool(name="xtp", bufs=1))
        xT_sb = [xtp.tile([128, S], bf16, tag=f"xt{k}", name=f"xt{k}")
                 for k in range(6)]
        w1_sb = [consts.tile([128, 256], bf16, tag=f"w1_{k}", name=f"w1s{k}")
                 for k in range(6)]
        w2_sb = [consts.tile([128, 256], bf16, tag=f"w2_{k}", name=f"w2s{k}")
                 for k in range(6)]
        wv_sb = [consts.tile([128, 192], bf16, tag=f"wv_{k}", name=f"wvs{k}")
                 for k in range(6)]
        for k in range(6):
            nc.sync.dma_start(out=w1_sb[k], in_=w1[k * 128:(k + 1) * 128, :])
            nc.sync.dma_start(out=w2_sb[k], in_=w2[k * 128:(k + 1) * 128, :])
            nc.sync.dma_start(out=wv_sb[k], in_=wv[k * 128:(k + 1) * 128, :])
        # xT chunk-wise, n-major so stage A's first chunk is ready early
        for n in range(4):
            for k in range(6):
                nc.sync.dma_start(
                    out=xT_sb[k][:, 512 * n:512 * (n + 1)],
                    in_=xT[k * 128:(k + 1) * 128, 512 * n:512 * (n + 1)])
        wo1_sb = consts.tile([128, D], bf16, tag="wo1")
        wo2_sb = consts.tile([64, D], bf16, tag="wo2")
        nc.sync.dma_start(out=wo1_sb, in_=wo1)
        nc.sync.dma_start(out=wo2_sb, in_=wo2)
        btA_sb = consts.tile([128, 1], f32, tag="btA")
        btB_sb = consts.tile([128, 1], f32, tag="btB")
        btC_sb = consts.tile([128, 1], f32, tag="btC")
        btD_sb = consts.tile([128, 1], f32, tag="btD")
        nc.sync.dma_start(out=btA_sb, in_=btA)
        nc.sync.dma_start(out=btB_sb, in_=btB)
        nc.sync.dma_start(out=btC_sb, in_=btC)
        nc.sync.dma_start(out=btD_sb, in_=btD)
        bvb_sb = consts.tile([128, 192], f32, tag="bvb")
        nc.sync.dma_start(out=bvb_sb, in_=bvb)
        triu_sb = consts.tile([128, 128], bf16, tag="triu")
        nc.sync.dma_start(out=triu_sb, in_=triu)
        onesb_sb = consts.tile([128, 64], bf16, tag="onesb")
        nc.sync.dma_start(out=onesb_sb, in_=onesb)
        onesr_sb = consts.tile([1, 64], f32r, tag="onesr")
        nc.sync.dma_start(out=onesr_sb, in_=onesr)

        # ---- long-lived activation tiles (bf16) ----
        qAB = qkv.tile([128, S], bf16, tag="qAB")  # [qT_h0|qT_h1], q-permuted
        kAB = qkv.tile([128, S], bf16, tag="kAB")  # [kT_h0|kT_h1], natural
        qC2 = qkv.tile([128, S], bf16, tag="qC2")  # qT_h2 dup'd, permuted
        kC2 = qkv.tile([128, S], bf16, tag="kC2")  # kT_h2 dup'd, natural
        v_sb = [qkv.tile([128, 195], bf16, tag=f"v{i}", name=f"vsb{i}")
                for i in range(NT)]
        attn01 = qkv.tile([128, S], bf16, tag="attn01")  # h0 parts 0-63, h1 64-127
        attn2 = qkv.tile([64, S], bf16, tag="attn2")

        def permuted_copy(dst, ps, n, bias):
            """psum 512-span n -> dst cols with even/odd tile permutation."""
            pr3 = ps.rearrange("p (c two k) -> p c two k", two=2, k=128)
            nc.vector.tensor_scalar_add(
                out=dst[:, 256 * n:256 * n + 256].rearrange("p (c k) -> p c k", k=128),
                in0=pr3[:, :, 0, :], scalar1=bias)
            nc.vector.tensor_scalar_add(
                out=dst[:, 1024 + 256 * n:1024 + 256 * n + 256].rearrange(
                    "p (c k) -> p c k", k=128),
                in0=pr3[:, :, 1, :], scalar1=bias)

        # ---- stage A for h0/h1 + V for all heads ----
        with tc.tile_pool(name="psA", bufs=2, space="PSUM") as psA:
            for n in range(4):
                xn = [xT_sb[k][:, 512 * n:512 * (n + 1)] for k in range(6)]
                psa = psA.tile([128, 512], f32, tag="psA")
                for k in range(6):
                    nc.tensor.matmul(psa, w1_sb[k][:, 0:128], xn[k],
                                     start=(k == 0), stop=(k == 5))
                permuted_copy(qAB, psa, n, btA_sb)
                psb = psA.tile([128, 512], f32, tag="psA")
                for k in range(6):
                    nc.tensor.matmul(psb, w1_sb[k][:, 128:256], xn[k],
                                     start=(k == 0), stop=(k == 5))
                nc.vector.tensor_scalar_add(
                    out=kAB[:, 512 * n:512 * (n + 1)], in0=psb, scalar1=btB_sb)

            for st in range(NT):
                psv = psA.tile([128, 192], f32, tag="psv")
                for k in range(6):
                    nc.tensor.matmul(
                        psv, xT_sb[k][:, 128 * st:128 * (st + 1)],
                        wv_sb[k], start=(k == 0), stop=(k == 5))
                vt = v_sb[st]
                nc.vector.tensor_tensor(
                    out=vt.rearrange("p (h c) -> p h c", c=65)[:, :, 0:64],
                    in0=psv.rearrange("p (h c) -> p h c", c=64),
                    in1=bvb_sb.rearrange("p (h c) -> p h c", c=64),
                    op=mybir.AluOpType.add)
                nc.vector.tensor_copy(
                    out=vt.rearrange("p (h c) -> p h c", c=65)[:, :, 64:65],
                    in_=onesb_sb[:, 0:3].unsqueeze(2))

        # ---- stage B pools (+ filler projection pool for h2) ----
        triu_b = triu_sb.unsqueeze(1)

        with tc.tile_pool(name="sc", bufs=2, space="PSUM") as scp, \
             tc.tile_pool(name="po", bufs=3, space="PSUM") as pop, \
             tc.tile_pool(name="aps", bufs=1, space="PSUM") as aps, \
             tc.tile_pool(name="pt", bufs=6) as ptp, \
             tc.tile_pool(name="nrm", bufs=4) as nrm:

            def norm(po, dst_rows, dst_cols):
                """po [65,512]: rows 0-63 = out^T, row 64 = denom.
                dst = attn tile slice [64, 512]."""
                den = nrm.tile([1, 512], f32r, tag="den")
                nc.vector.tensor_copy(out=den, in_=po[64:65, :])
                dps = pop.tile([64, 512], f32, tag="po")
                nc.tensor.matmul(dps, onesr_sb, den, start=True, stop=True)
                rec = nrm.tile([64, 512], f32, tag="rec")
                nc.vector.reciprocal_approx_fast(out=rec, in_=dps)
                nc.vector.tensor_tensor(
                    out=dst_rows[:, dst_cols], in0=po[0:64, :], in1=rec, op=mult)

            def exp_unit(sc, mask_lo, mask_hi):
                pt = ptp.tile([128, 1024], bf16, tag="pt")
                nc.scalar.activation(out=pt, in_=sc, func=Exp, scale=0.125)
                if mask_lo and mask_hi:
                    p3 = pt.rearrange("p (c k) -> p c k", k=128)
                    nc.vector.tensor_tensor(
                        out=p3, in0=p3, in1=triu_b.broadcast_to([128, 8, 128]),
                        op=mult)
                elif mask_lo or mask_hi:
                    off = 0 if mask_lo else 512
                    p3 = pt[:, off:off + 512].rearrange("p (c k) -> p c k", k=128)
                    nc.vector.tensor_tensor(
                        out=p3, in0=p3, in1=triu_b.broadcast_to([128, 4, 128]),
                        op=mult)
                return pt

            # ---- B01: heads h0,h1 packed on partition halves ----
            def b01(g, hh):
                qc = slice(1024 * g + 512 * hh, 1024 * g + 512 * hh + 512)
                po0 = pop.tile([65, 512], f32, tag="po")
                po1 = pop.tile([65, 512], f32, tag="po")
                for w in range(NW):
                    klo = slice(WIN * w, WIN * w + 128)
                    sc = scp.tile([128, 1024], f32, tag="sc")
                    nc.tensor.matmul(sc[:, 0:512], kAB[0:64, klo],
                                     qAB[0:64, qc], start=True, stop=True)
                    nc.tensor.matmul(sc[:, 512:1024], kAB[64:128, klo],
                                     qAB[64:128, qc], start=True, stop=True)
                    pt = exp_unit(sc, mask_lo=(g == 0), mask_hi=(g == 0))
                    last = (g == 0 and w == NW - 1)
                    nc.tensor.matmul(po0, v_sb[2 * w][:, 0:65],
                                     pt[:, 0:512], start=(w == 0), stop=last)
                    nc.tensor.matmul(po1, v_sb[2 * w][:, 65:130],
                                     pt[:, 512:1024], start=(w == 0), stop=last)
                    if g == 1:
                        khi = slice(WIN * w + 128, WIN * w + 256)
                        sch = scp.tile([128, 1024], f32, tag="sc")
                        nc.tensor.matmul(sch[:, 0:512], kAB[0:64, khi],
                                         qAB[0:64, qc], start=True, stop=True)
                        nc.tensor.matmul(sch[:, 512:1024], kAB[64:128, khi],
                                         qAB[64:128, qc], start=True, stop=True)
                        pth = exp_unit(sch, mask_lo=True, mask_hi=True)
                        last = (w == NW - 1)
                        nc.tensor.matmul(po0, v_sb[2 * w + 1][:, 0:65],
                                         pth[:, 0:512], start=False, stop=last)
                        nc.tensor.matmul(po1, v_sb[2 * w + 1][:, 65:130],
                                         pth[:, 512:1024], start=False, stop=last)
                cols = slice(1024 * g + 512 * hh, 1024 * g + 512 * hh + 512)
                norm(po0, attn01[0:64, :], cols)
                norm(po1, attn01[64:128, :], cols)

            for g in range(2):
                for hh in range(2):
                    b01(g, hh)

            # ---- filler: stage A for h2 (emitted after B01 -> lower priority,
            # fills PE idle while ACT works through B01's exps) ----
            for n in range(4):
                xn = [xT_sb[k][:, 512 * n:512 * (n + 1)] for k in range(6)]
                psq = aps.tile([128, 512], f32, tag="apsA")
                for k in range(6):
                    nc.tensor.matmul(psq, w2_sb[k][:, 0:128], xn[k],
                                     start=(k == 0), stop=(k == 5))
                permuted_copy(qC2, psq, n, btC_sb)
            for n in range(4):
                xn = [xT_sb[k][:, 512 * n:512 * (n + 1)] for k in range(6)]
                psk = aps.tile([128, 512], f32, tag="apsA")
                for k in range(6):
                    nc.tensor.matmul(psk, w2_sb[k][:, 128:256], xn[k],
                                     start=(k == 0), stop=(k == 5))
                nc.vector.tensor_scalar_add(
                    out=kC2[:, 512 * n:512 * (n + 1)], in0=psk, scalar1=btD_sb)

            # ---- B2: head h2, groups g0/g1 packed on partition halves ----
            def b2(hh):
                q0 = slice(512 * hh, 512 * hh + 512)            # grp0 cols
                q1 = slice(1024 + 512 * hh, 1024 + 512 * hh + 512)  # grp1 cols
                pg0 = pop.tile([65, 512], f32, tag="po")
                pg1 = pop.tile([65, 512], f32, tag="po")
                for w in range(NW):
                    klo = slice(WIN * w, WIN * w + 128)
                    khi = slice(WIN * w + 128, WIN * w + 256)
                    sc = scp.tile([128, 1024], f32, tag="sc")
                    nc.tensor.matmul(sc[:, 0:512], kC2[0:64, klo],
                                     qC2[0:64, q0], start=True, stop=True)
                    nc.tensor.matmul(sc[:, 512:1024], kC2[64:128, klo],
                                     qC2[64:128, q1], start=True, stop=True)
                    pt = exp_unit(sc, mask_lo=True, mask_hi=False)
                    nc.tensor.matmul(pg0, v_sb[2 * w][:, 130:195],
                                     pt[:, 0:512], start=(w == 0),
                                     stop=(w == NW - 1))
                    nc.tensor.matmul(pg1, v_sb[2 * w][:, 130:195],
                                     pt[:, 512:1024], start=(w == 0), stop=False)
                    scb = scp.tile([128, 512], f32, tag="sc")
                    nc.tensor.matmul(scb, kC2[0:64, khi], qC2[0:64, q1],
                                     start=True, stop=True)
                    ptb = ptp.tile([128, 512], bf16, tag="pt")
                    nc.scalar.activation(out=ptb, in_=scb, func=Exp, scale=0.125)
                    p3 = ptb.rearrange("p (c k) -> p c k", k=128)
                    nc.vector.tensor_tensor(
                        out=p3, in0=p3, in1=triu_b.broadcast_to([128, 4, 128]),
                        op=mult)
                    nc.tensor.matmul(pg1, v_sb[2 * w + 1][:, 130:195],
                                     ptb, start=False, stop=(w == NW - 1))
                norm(pg0, attn2, slice(512 * hh, 512 * hh + 512))
                norm(pg1, attn2, slice(1024 + 512 * hh, 1024 + 512 * hh + 512))

            for hh in range(2):
                b2(hh)

        # ---- stage C ----
        with tc.tile_pool(name="oc", bufs=3, space="PSUM") as oc_pool, \
             tc.tile_pool(name="ost", bufs=3) as ost_pool:
            for p in range(NT):
                pso = oc_pool.tile([128, D], f32, tag="pso")
                pcols = slice(128 * p, 128 * (p + 1))
                for (n0, n1) in ((0, 512), (512, 768)):
                    nc.tensor.matmul(pso[:, n0:n1], attn01[:, pcols],
                                     wo1_sb[:, n0:n1], start=True, stop=False)
                    nc.tensor.matmul(pso[:, n0:n1], attn2[:, pcols],
                                     wo2_sb[:, n0:n1], start=False, stop=True)
                ot = ost_pool.tile([128, D], bf16, tag="ot")
                nc.vector.tensor_copy(out=ot, in_=pso)
                t = 2 * p if p < 8 else 2 * (p - 8) + 1
                nc.sync.dma_start(out=out[128 * t:128 * (t + 1), :], in_=ot)

    nc.compile()
    return nc


def _prep_core_inputs(inputs, c):
    x = inputs["x"]
    Wq, bq = inputs["Wq"], inputs["bq"]
    Wk, bk = inputs["Wk"], inputs["bk"]
    Wv, bv = inputs["Wv"], inputs["bv"]
    Wo = inputs["Wo"]
    b = c // 4
    r0 = (c % 4) * DH  # first feature row of this core's 192-row head block

    xT = np.ascontiguousarray(np.asarray(x[b]).T).astype(BF16)
    W1 = np.concatenate(
        [Wq[r0:r0 + 128].T, Wk[r0:r0 + 128].T], axis=1).astype(BF16)
    q2 = Wq[r0 + 128:r0 + 192].T
    k2 = Wk[r0 + 128:r0 + 192].T
    W2 = np.concatenate([q2, q2, k2, k2], axis=1).astype(BF16)
    Wvp = np.ascontiguousarray(Wv[r0:r0 + 192].T)
    bq2 = bq[r0 + 128:r0 + 192]
    bk2 = bk[r0 + 128:r0 + 192]
    bvb = np.zeros((128, 192), np.float32)
    for h in range(NHC):
        bvb[:, 64 * h:64 * h + 64] = bv[r0 + 64 * h:r0 + 64 * (h + 1)][None, :]

    return dict(
        xT=xT, w1=W1, w2=W2, wv=Wvp.astype(BF16),
        wo1=np.ascontiguousarray(Wo[:, r0:r0 + 128].T).astype(BF16),
        wo2=np.ascontiguousarray(Wo[:, r0 + 128:r0 + 192].T).astype(BF16),
        btA=np.ascontiguousarray(bq[r0:r0 + 128].reshape(128, 1)).astype(np.float32),
        btB=np.ascontiguousarray(bk[r0:r0 + 128].reshape(128, 1)).astype(np.float32),
        btC=np.concatenate([bq2, bq2]).reshape(128, 1).astype(np.float32),
        btD=np.concatenate([bk2, bk2]).reshape(128, 1).astype(np.float32),
        bvb=np.ascontiguousarray(bvb),
        triu=np.triu(np.ones((128, 128), np.float32)).astype(BF16),
        onesb=np.ones((128, 64), np.float32).astype(BF16),
        onesr=np.ones((1, 64), np.float32),
    )


def _install_ntff_hook():
    """Register antenv.axon_hooks with a ctypes NTFF profile hook so
    run_bass_kernel_spmd(trace=True) can capture device-side exec time."""
    import types, ctypes, contextlib

    try:
        import antenv.axon_hooks  # noqa: F401
        return
    except ImportError:
        pass
    so_path = "/opt/axon/libaxon_pjrt.so"
    lib = ctypes.CDLL(so_path)
    if not hasattr(lib, "axon_start_nrt_profile"):
        return
    lib.axon_start_nrt_profile.argtypes = [
        ctypes.POINTER(ctypes.c_int64), ctypes.c_size_t]
    lib.axon_start_nrt_profile.restype = ctypes.c_int64
    lib.axon_stop_nrt_profile.argtypes = [ctypes.c_char_p]
    lib.axon_stop_nrt_profile.restype = ctypes.c_int64

    @contextlib.contextmanager
    def _hook(output_dir, device_ids):
        import jax
        jax.devices()
        if device_ids:
            ids = (ctypes.c_int64 * len(device_ids))(*device_ids)
            rc = lib.axon_start_nrt_profile(ids, len(device_ids))
        else:
            rc = lib.axon_start_nrt_profile(None, 0)
        if rc != 0:
            raise RuntimeError(f"axon_start_nrt_profile rc={rc}")
        try:
            yield
        finally:
            n = lib.axon_stop_nrt_profile(str(output_dir).encode())
            print(f"profile: {n} file(s) written to {output_dir}")

    mod = types.ModuleType("antenv.axon_hooks")
    mod.get_axon_ntff_profile_hook = lambda: _hook
    mod.set_axon_ntff_profile_hook = lambda h: None
    sys.modules["antenv.axon_hooks"] = mod
    import antenv
    antenv.axon_hooks = mod


def kernel(**inputs):
    import os
    from concourse import bass_utils

    if "nc" not in _CACHE:
        _CACHE["nc"] = _build_program()
    nc = _CACHE["nc"]

    trace = bool(os.environ.get("MHA_TRACE"))
    kwargs = {}
    if trace:
        _install_ntff_hook()
        kwargs = dict(trace=True, tmpdir="/tmp/mha_trace")
        os.makedirs("/tmp/mha_trace", exist_ok=True)

    in_maps = [_prep_core_inputs(inputs, c) for c in range(8)]
    res = bass_utils.run_bass_kernel_spmd(
        nc, in_maps, core_ids=list(range(8)), **kwargs)
    _CACHE["last_results"] = res
    if trace and res.exec_time_ns is not None:
        print(f"HW exec time: {res.exec_time_ns} ns")
    out = np.zeros((B, S, D), np.float32)
    for c in range(8):
        out[c // 4] += res.results[c]["out"].astype(np.float32)
    out += np.asarray(inputs["bo"], np.float32).reshape(1, 1, D)
    return out


# revision 17
# speedup vs baseline: 1.5081x; 1.2702x over previous
"""Trainium2 Bass kernel for nn_MultiHeadAttention_824633721543.

MHA with periodic prefix mask: allowed iff (q % 256) >= (k % 256).
B=2, S=2048, D=768, H=12, Dk=64, WINDOW=256.

Sharding: 8 cores = 2 batches x 4 head-groups (3 heads each). Each core
computes q/k/v projections for its heads, the masked softmax attention, and
a partial O-projection; the host sums the 4 partials per batch and adds bo.

v2 design (all activations/weights bf16, PSUM accumulation fp32):
  - scores computed as S^T [k,q]; q columns tile-permuted (even 128-tiles |
    odd 128-tiles) so masks reduce to one shared 128x128 triu tile.
  - h0/h1 q,k stacked on partition halves of one [128,S] tile; score matmuls
    for both heads issued as K=64 row-tiled pairs (tile_position (0,0)/(64,0))
    that run concurrently in the PE array.
  - h2 q,k duplicated onto both partition halves (free: the projection
    matmul's stationary has spare M), so h2's two q-groups pack the same way.
  - exp on ACT (scale=1/8), bf16 out; mask = DVE bf16 multiply (4x mode).
  - P@V with [V|1] stationary -> out^T plus denominator row, accumulated over
    windows in PSUM; normalization via K=1 broadcast matmul + DVE.
  - stage A for h2 is emitted AFTER stage B of h0/h1 so the Tile scheduler
    uses it as PE filler while ACT chews exp (keeps the PE HAM-warm).
  - O-projection: h0/h1 as one K=128 matmul, h2 K=64; output DMA'd as bf16.
"""

import sys

sys.path.insert(0, "/opt/trn_rl_repo")

import numpy as np
import ml_dtypes

BF16 = ml_dtypes.bfloat16

B = 2
S = 2048
D = 768
DK = 64
WIN = 256
NW = S // WIN   # 8 windows
NHC = 3         # heads per core
DH = NHC * DK   # 192
NT = S // 128   # 16 q tiles

_CACHE = {}


def _build_program():
    import concourse.tile as tile
    from concourse import mybir, bacc
    from contextlib import ExitStack

    f32 = mybir.dt.float32
    f32r = mybir.dt.float32r
    bf16 = mybir.dt.bfloat16
    Exp = mybir.ActivationFunctionType.Exp
    mult = mybir.AluOpType.mult

    nc = bacc.Bacc("TRN2", target_bir_lowering=False, debug=False)

    # xB: [128, 12288] packed as (n 4, k 6, s 512) so each of the 4 DMAs moves
    # 6KB-per-partition lines; wP: [128, 4224] = per k-chunk [w1 256|w2 256|wv 192]
    xB = nc.dram_tensor("xB", [128, 12288], bf16, kind="ExternalInput").ap()
    wP = nc.dram_tensor("wP", [128, 4224], bf16, kind="ExternalInput").ap()
    wo1 = nc.dram_tensor("wo1", [128, D], bf16, kind="ExternalInput").ap()
    wo2 = nc.dram_tensor("wo2", [64, D], bf16, kind="ExternalInput").ap()
    btA = nc.dram_tensor("btA", [128, 1], f32, kind="ExternalInput").ap()
    btB = nc.dram_tensor("btB", [128, 1], f32, kind="ExternalInput").ap()
    btC = nc.dram_tensor("btC", [128, 1], f32, kind="ExternalInput").ap()
    btD = nc.dram_tensor("btD", [128, 1], f32, kind="ExternalInput").ap()
    bvb = nc.dram_tensor("bvb", [128, 192], f32, kind="ExternalInput").ap()
    triu = nc.dram_tensor("triu", [128, 128], bf16, kind="ExternalInput").ap()
    onesb = nc.dram_tensor("onesb", [128, 64], bf16, kind="ExternalInput").ap()
    onesr = nc.dram_tensor("onesr", [1, 64], f32r, kind="ExternalInput").ap()
    out = nc.dram_tensor("out", [S, D], bf16, kind="ExternalOutput").ap()

    with tile.TileContext(nc) as tc, ExitStack() as ctx:
        consts = ctx.enter_context(tc.tile_pool(name="consts", bufs=1))
        qkv = ctx.enter_context(tc.tile_pool(name="qkv", bufs=1))

        xtp = ctx.enter_context(tc.tile_pool(name="xtp", bufs=1))
        xB_sb = xtp.tile([128, 12288], bf16, tag="xB")
        wP_sb = consts.tile([128, 4224], bf16, tag="wP")
        nc.sync.dma_start(out=wP_sb, in_=wP)
        for n in range(4):
            nc.sync.dma_start(out=xB_sb[:, 3072 * n:3072 * (n + 1)],
                              in_=xB[:, 3072 * n:3072 * (n + 1)])

        def xn_sl(k, n):
            return xB_sb[:, 3072 * n + 512 * k:3072 * n + 512 * k + 512]

        def xst_sl(k, st):
            c = 3072 * (st // 4) + 512 * k + 128 * (st % 4)
            return xB_sb[:, c:c + 128]

        w1_sb = [wP_sb[:, 704 * k:704 * k + 256] for k in range(6)]
        w2_sb = [wP_sb[:, 704 * k + 256:704 * k + 512] for k in range(6)]
        wv_sb = [wP_sb[:, 704 * k + 512:704 * k + 704] for k in range(6)]
        wo1_sb = consts.tile([128, D], bf16, tag="wo1")
        wo2_sb = consts.tile([64, D], bf16, tag="wo2")
        nc.sync.dma_start(out=wo1_sb, in_=wo1)
        nc.sync.dma_start(out=wo2_sb, in_=wo2)
        btA_sb = consts.tile([128, 1], f32, tag="btA")
        btB_sb = consts.tile([128, 1], f32, tag="btB")
        btC_sb = consts.tile([128, 1], f32, tag="btC")
        btD_sb = consts.tile([128, 1], f32, tag="btD")
        nc.sync.dma_start(out=btA_sb, in_=btA)
        nc.sync.dma_start(out=btB_sb, in_=btB)
        nc.sync.dma_start(out=btC_sb, in_=btC)
        nc.sync.dma_start(out=btD_sb, in_=btD)
        bvb_sb = consts.tile([128, 192], f32, tag="bvb")
        nc.sync.dma_start(out=bvb_sb, in_=bvb)
        triu_sb = consts.tile([128, 128], bf16, tag="triu")
        nc.sync.dma_start(out=triu_sb, in_=triu)
        onesb_sb = consts.tile([128, 64], bf16, tag="onesb")
        nc.sync.dma_start(out=onesb_sb, in_=onesb)
        onesr_sb = consts.tile([1, 64], f32r, tag="onesr")
        nc.sync.dma_start(out=onesr_sb, in_=onesr)

        # ---- long-lived activation tiles (bf16) ----
        qAB = qkv.tile([128, S], bf16, tag="qAB")  # [qT_h0|qT_h1], q-permuted
        kAB = qkv.tile([128, S], bf16, tag="kAB")  # [kT_h0|kT_h1], natural
        qC2 = qkv.tile([128, S], bf16, tag="qC2")  # qT_h2 dup'd, permuted
        kC2 = qkv.tile([128, S], bf16, tag="kC2")  # kT_h2 dup'd, natural
        v_sb = [qkv.tile([128, 195], bf16, tag=f"v{i}", name=f"vsb{i}")
                for i in range(NT)]
        attn01 = qkv.tile([128, S], bf16, tag="attn01")  # h0 parts 0-63, h1 64-127
        attn2 = qkv.tile([64, S], bf16, tag="attn2")

        def permuted_copy(dst, ps, n, bias):
            """psum 512-span n -> dst cols with even/odd tile permutation."""
            pr3 = ps.rearrange("p (c two k) -> p c two k", two=2, k=128)
            nc.vector.tensor_scalar_add(
                out=dst[:, 256 * n:256 * n + 256].rearrange("p (c k) -> p c k", k=128),
                in0=pr3[:, :, 0, :], scalar1=bias)
            nc.vector.tensor_scalar_add(
                out=dst[:, 1024 + 256 * n:1024 + 256 * n + 256].rearrange(
                    "p (c k) -> p c k", k=128),
                in0=pr3[:, :, 1, :], scalar1=bias)

        # ---- stage A for h0/h1 + V for all heads ----
        with tc.tile_pool(name="psA", bufs=2, space="PSUM") as psA:
            for n in range(4):
                xn = [xn_sl(k, n) for k in range(6)]
                psa = psA.tile([128, 512], f32, tag="psA")
                for k in range(6):
                    nc.tensor.matmul(psa, w1_sb[k][:, 0:128], xn[k],
                                     start=(k == 0), stop=(k == 5))
                permuted_copy(qAB, psa, n, btA_sb)
                psb = psA.tile([128, 512], f32, tag="psA")
                for k in range(6):
                    nc.tensor.matmul(psb, w1_sb[k][:, 128:256], xn[k],
                                     start=(k == 0), stop=(k == 5))
                nc.vector.tensor_scalar_add(
                    out=kAB[:, 512 * n:512 * (n + 1)], in0=psb, scalar1=btB_sb)

            for st in range(NT):
                psv = psA.tile([128, 192], f32, tag="psv")
                for k in range(6):
                    nc.tensor.matmul(
                        psv, xst_sl(k, st),
                        wv_sb[k], start=(k == 0), stop=(k == 5))
                vt = v_sb[st]
                nc.vector.tensor_tensor(
                    out=vt.rearrange("p (h c) -> p h c", c=65)[:, :, 0:64],
                    in0=psv.rearrange("p (h c) -> p h c", c=64),
                    in1=bvb_sb.rearrange("p (h c) -> p h c", c=64),
                    op=mybir.AluOpType.add)
                nc.vector.tensor_copy(
                    out=vt.rearrange("p (h c) -> p h c", c=65)[:, :, 64:65],
                    in_=onesb_sb[:, 0:3].unsqueeze(2))

        # ---- stage B pools (+ filler projection pool for h2) ----
        triu_b = triu_sb.unsqueeze(1)

        with tc.tile_pool(name="sc", bufs=2, space="PSUM") as scp, \
             tc.tile_pool(name="po", bufs=3, space="PSUM") as pop, \
             tc.tile_pool(name="aps", bufs=1, space="PSUM") as aps, \
             tc.tile_pool(name="pt", bufs=6) as ptp, \
             tc.tile_pool(name="nrm", bufs=4) as nrm:

            def norm(po, dst_rows, dst_cols):
                """po [65,512]: rows 0-63 = out^T, row 64 = denom.
                dst = attn tile slice [64, 512]."""
                den = nrm.tile([1, 512], f32r, tag="den")
                nc.vector.tensor_copy(out=den, in_=po[64:65, :])
                dps = pop.tile([64, 512], f32, tag="po")
                nc.tensor.matmul(dps, onesr_sb, den, start=True, stop=True)
                rec = nrm.tile([64, 512], f32, tag="rec")
                nc.vector.reciprocal_approx_fast(out=rec, in_=dps)
                nc.vector.tensor_tensor(
                    out=dst_rows[:, dst_cols], in0=po[0:64, :], in1=rec, op=mult)

            def exp_unit(sc, mask_lo, mask_hi):
                pt = ptp.tile([128, 1024], bf16, tag="pt")
                nc.scalar.activation(out=pt, in_=sc, func=Exp, scale=0.125)
                if mask_lo and mask_hi:
                    p3 = pt.rearrange("p (c k) -> p c k", k=128)
                    nc.vector.tensor_tensor(
                        out=p3, in0=p3, in1=triu_b.broadcast_to([128, 8, 128]),
                        op=mult)
                elif mask_lo or mask_hi:
                    off = 0 if mask_lo else 512
                    p3 = pt[:, off:off + 512].rearrange("p (c k) -> p c k", k=128)
                    nc.vector.tensor_tensor(
                        out=p3, in0=p3, in1=triu_b.broadcast_to([128, 4, 128]),
                        op=mult)
                return pt

            # ---- B01: heads h0,h1 packed on partition halves ----
            def b01(g, hh):
                qc = slice(1024 * g + 512 * hh, 1024 * g + 512 * hh + 512)
                po0 = pop.tile([65, 512], f32, tag="po")
                po1 = pop.tile([65, 512], f32, tag="po")
                for w in range(NW):
                    klo = slice(WIN * w, WIN * w + 128)
                    sc = scp.tile([128, 1024], f32, tag="sc")
                    nc.tensor.matmul(sc[:, 0:512], kAB[0:64, klo],
                                     qAB[0:64, qc], start=True, stop=True)
                    nc.tensor.matmul(sc[:, 512:1024], kAB[64:128, klo],
                                     qAB[64:128, qc], start=True, stop=True)
                    pt = exp_unit(sc, mask_lo=(g == 0), mask_hi=(g == 0))
                    last = (g == 0 and w == NW - 1)
                    nc.tensor.matmul(po0, v_sb[2 * w][:, 0:65],
                                     pt[:, 0:512], start=(w == 0), stop=last)
                    nc.tensor.matmul(po1, v_sb[2 * w][:, 65:130],
                                     pt[:, 512:1024], start=(w == 0), stop=last)
                    if g == 1:
                        khi = slice(WIN * w + 128, WIN * w + 256)
                        sch = scp.tile([128, 1024], f32, tag="sc")
                        nc.tensor.matmul(sch[:, 0:512], kAB[0:64, khi],
                                         qAB[0:64, qc], start=True, stop=True)
                        nc.tensor.matmul(sch[:, 512:1024], kAB[64:128, khi],
                                         qAB[64:128, qc], start=True, stop=True)
                        pth = exp_unit(sch, mask_lo=True, mask_hi=True)
                        last = (w == NW - 1)
                        nc.tensor.matmul(po0, v_sb[2 * w + 1][:, 0:65],
                                         pth[:, 0:512], start=False, stop=last)
                        nc.tensor.matmul(po1, v_sb[2 * w + 1][:, 65:130],
                                         pth[:, 512:1024], start=False, stop=last)
                cols = slice(1024 * g + 512 * hh, 1024 * g + 512 * hh + 512)
                norm(po0, attn01[0:64, :], cols)
                norm(po1, attn01[64:128, :], cols)

            for g in range(2):
                for hh in range(2):
                    b01(g, hh)

            # ---- filler: stage A for h2 (emitted after B01 -> lower priority,
            # fills PE idle while ACT works through B01's exps) ----
            for n in range(4):
                xn = [xn_sl(k, n) for k in range(6)]
                psq = aps.tile([128, 512], f32, tag="apsA")
                for k in range(6):
                    nc.tensor.matmul(psq, w2_sb[k][:, 0:128], xn[k],
                                     start=(k == 0), stop=(k == 5))
                permuted_copy(qC2, psq, n, btC_sb)
            for n in range(4):
                xn = [xn_sl(k, n) for k in range(6)]
                psk = aps.tile([128, 512], f32, tag="apsA")
                for k in range(6):
                    nc.tensor.matmul(psk, w2_sb[k][:, 128:256], xn[k],
                                     start=(k == 0), stop=(k == 5))
                nc.vector.tensor_scalar_add(
                    out=kC2[:, 512 * n:512 * (n + 1)], in0=psk, scalar1=btD_sb)

            # ---- B2: head h2, groups g0/g1 packed on partition halves ----
            def b2(hh):
                q0 = slice(512 * hh, 512 * hh + 512)            # grp0 cols
                q1 = slice(1024 + 512 * hh, 1024 + 512 * hh + 512)  # grp1 cols
                pg0 = pop.tile([65, 512], f32, tag="po")
                pg1 = pop.tile([65, 512], f32, tag="po")
                for w in range(NW):
                    klo = slice(WIN * w, WIN * w + 128)
                    khi = slice(WIN * w + 128, WIN * w + 256)
                    sc = scp.tile([128, 1024], f32, tag="sc")
                    nc.tensor.matmul(sc[:, 0:512], kC2[0:64, klo],
                                     qC2[0:64, q0], start=True, stop=True)
                    nc.tensor.matmul(sc[:, 512:1024], kC2[64:128, klo],
                                     qC2[64:128, q1], start=True, stop=True)
                    pt = exp_unit(sc, mask_lo=True, mask_hi=False)
                    nc.tensor.matmul(pg0, v_sb[2 * w][:, 130:195],
                                     pt[:, 0:512], start=(w == 0),
                                     stop=(w == NW - 1))
                    nc.tensor.matmul(pg1, v_sb[2 * w][:, 130:195],
                                     pt[:, 512:1024], start=(w == 0), stop=False)
                    scb = scp.tile([128, 512], f32, tag="sc")
                    nc.tensor.matmul(scb, kC2[0:64, khi], qC2[0:64, q1],
                                     start=True, stop=True)
                    ptb = ptp.tile([128, 512], bf16, tag="pt")
                    nc.scalar.activation(out=ptb, in_=scb, func=Exp, scale=0.125)
                    p3 = ptb.rearrange("p (c k) -> p c k", k=128)
                    nc.vector.tensor_tensor(
                        out=p3, in0=p3, in1=triu_b.broadcast_to([128, 4, 128]),
                        op=mult)
                    nc.tensor.matmul(pg1, v_sb[2 * w + 1][:, 130:195],
                                     ptb, start=False, stop=(w == NW - 1))
                norm(pg0, attn2, slice(512 * hh, 512 * hh + 512))
                norm(pg1, attn2, slice(1024 + 512 * hh, 1024 + 512 * hh + 512))

            for hh in range(2):
                b2(hh)

        # ---- stage C ----
        with tc.tile_pool(name="oc", bufs=3, space="PSUM") as oc_pool, \
             tc.tile_pool(name="ost", bufs=3) as ost_pool:
            for p in range(NT):
                pso = oc_pool.tile([128, D], f32, tag="pso")
                pcols = slice(128 * p, 128 * (p + 1))
                for (n0, n1) in ((0, 512), (512, 768)):
                    nc.tensor.matmul(pso[:, n0:n1], attn01[:, pcols],
                                     wo1_sb[:, n0:n1], start=True, stop=False)
                    nc.tensor.matmul(pso[:, n0:n1], attn2[:, pcols],
                                     wo2_sb[:, n0:n1], start=False, stop=True)
                ot = ost_pool.tile([128, D], bf16, tag="ot")
                nc.scalar.copy(out=ot, in_=pso)
                t = 2 * p if p < 8 else 2 * (p - 8) + 1
                nc.sync.dma_start(out=out[128 * t:128 * (t + 1), :], in_=ot)

    nc.compile()
    return nc


def _prep_core_inputs(inputs, c):
    x = inputs["x"]
    Wq, bq = inputs["Wq"], inputs["bq"]
    Wk, bk = inputs["Wk"], inputs["bk"]
    Wv, bv = inputs["Wv"], inputs["bv"]
    Wo = inputs["Wo"]
    b = c // 4
    r0 = (c % 4) * DH  # first feature row of this core's 192-row head block

    xT = np.asarray(x[b]).T.astype(np.float32)  # [768, 2048]
    W1 = np.concatenate(
        [Wq[r0:r0 + 128].T, Wk[r0:r0 + 128].T], axis=1)
    q2 = Wq[r0 + 128:r0 + 192].T
    k2 = Wk[r0 + 128:r0 + 192].T
    W2 = np.concatenate([q2, q2, k2, k2], axis=1)
    Wvp = Wv[r0:r0 + 192].T
    # packed x: [128, (n 4, k 6, s 512)]
    xBp = np.zeros((128, 12288), np.float32)
    for n in range(4):
        for k in range(6):
            xBp[:, 3072 * n + 512 * k:3072 * n + 512 * (k + 1)] = \
                xT[128 * k:128 * (k + 1), 512 * n:512 * (n + 1)]
    # packed weights: [128, (k 6, [w1 256 | w2 256 | wv 192])]
    wPp = np.zeros((128, 4224), np.float32)
    for k in range(6):
        wPp[:, 704 * k:704 * k + 256] = W1[128 * k:128 * (k + 1)]
        wPp[:, 704 * k + 256:704 * k + 512] = W2[128 * k:128 * (k + 1)]
        wPp[:, 704 * k + 512:704 * k + 704] = Wvp[128 * k:128 * (k + 1)]
    bq2 = bq[r0 + 128:r0 + 192]
    bk2 = bk[r0 + 128:r0 + 192]
    bvb = np.zeros((128, 192), np.float32)
    for h in range(NHC):
        bvb[:, 64 * h:64 * h + 64] = bv[r0 + 64 * h:r0 + 64 * (h + 1)][None, :]

    return dict(
        xB=xBp.astype(BF16), wP=wPp.astype(BF16),
        wo1=np.ascontiguousarray(Wo[:, r0:r0 + 128].T).astype(BF16),
        wo2=np.ascontiguousarray(Wo[:, r0 + 128:r0 + 192].T).astype(BF16),
        btA=np.ascontiguousarray(bq[r0:r0 + 128].reshape(128, 1)).astype(np.float32),
        btB=np.ascontiguousarray(bk[r0:r0 + 128].reshape(128, 1)).astype(np.float32),
        btC=np.concatenate([bq2, bq2]).reshape(128, 1).astype(np.float32),
        btD=np.concatenate([bk2, bk2]).reshape(128, 1).astype(np.float32),
        bvb=np.ascontiguousarray(bvb),
        triu=np.triu(np.ones((128, 128), np.float32)).astype(BF16),
        onesb=np.ones((128, 64), np.float32).astype(BF16),
        onesr=np.ones((1, 64), np.float32),
    )


def _install_ntff_hook():
    """Register antenv.axon_hooks with a ctypes NTFF profile hook so
    run_bass_kernel_spmd(trace=True) can capture device-side exec time."""
    import types, ctypes, contextlib

    try:
        import antenv.axon_hooks  # noqa: F401
        return
    except ImportError:
        pass
    so_path = "/opt/axon/libaxon_pjrt.so"
    lib = ctypes.CDLL(so_path)
    if not hasattr(lib, "axon_start_nrt_profile"):
        return
    lib.axon_start_nrt_profile.argtypes = [
        ctypes.POINTER(ctypes.c_int64), ctypes.c_size_t]
    lib.axon_start_nrt_profile.restype = ctypes.c_int64
    lib.axon_stop_nrt_profile.argtypes = [ctypes.c_char_p]
    lib.axon_stop_nrt_profile.restype = ctypes.c_int64

    @contextlib.contextmanager
    def _hook(output_dir, device_ids):
        import jax
        jax.devices()
        if device_ids:
            ids = (ctypes.c_int64 * len(device_ids))(*device_ids)
            rc = lib.axon_start_nrt_profile(ids, len(device_ids))
        else:
            rc = lib.axon_start_nrt_profile(None, 0)
        if rc != 0:
            raise RuntimeError(f"axon_start_nrt_profile rc={rc}")
        try:
            yield
        finally:
            n = lib.axon_stop_nrt_profile(str(output_dir).encode())
            print(f"profile: {n} file(s) written to {output_dir}")

    mod = types.ModuleType("antenv.axon_hooks")
    mod.get_axon_ntff_profile_hook = lambda: _hook
    mod.set_axon_ntff_profile_hook = lambda h: None
    sys.modules["antenv.axon_hooks"] = mod
    import antenv
    antenv.axon_hooks = mod


def kernel(**inputs):
    import os
    from concourse import bass_utils

    if "nc" not in _CACHE:
        _CACHE["nc"] = _build_program()
    nc = _CACHE["nc"]

    trace = bool(os.environ.get("MHA_TRACE"))
    kwargs = {}
    if trace:
        _install_ntff_hook()
        kwargs = dict(trace=True, tmpdir="/tmp/mha_trace")
        os.makedirs("/tmp/mha_trace", exist_ok=True)

    in_maps = [_prep_core_inputs(inputs, c) for c in range(8)]
    res = bass_utils.run_bass_kernel_spmd(
        nc, in_maps, core_ids=list(range(8)), **kwargs)
    _CACHE["last_results"] = res
    if trace and res.exec_time_ns is not None:
        print(f"HW exec time: {res.exec_time_ns} ns")
    out = np.zeros((B, S, D), np.float32)
    for c in range(8):
        out[c // 4] += res.results[c]["out"].astype(np.float32)
    out += np.asarray(inputs["bo"], np.float32).reshape(1, 1, D)
    return out


# revision 22
# speedup vs baseline: 1.5673x; 1.0392x over previous
"""Trainium2 Bass kernel for nn_MultiHeadAttention_824633721543.

MHA with periodic prefix mask: allowed iff (q % 256) >= (k % 256).
B=2, S=2048, D=768, H=12, Dk=64, WINDOW=256.

Sharding: 8 cores = 2 batches x 4 head-groups (3 heads each). Each core
computes q/k/v projections for its heads, the masked softmax attention, and
a partial O-projection; the host sums the 4 partials per batch and adds bo.

v2 design (all activations/weights bf16, PSUM accumulation fp32):
  - scores computed as S^T [k,q]; q columns tile-permuted (even 128-tiles |
    odd 128-tiles) so masks reduce to one shared 128x128 triu tile.
  - h0/h1 q,k stacked on partition halves of one [128,S] tile; score matmuls
    for both heads issued as K=64 row-tiled pairs (tile_position (0,0)/(64,0))
    that run concurrently in the PE array.
  - h2 q,k duplicated onto both partition halves (free: the projection
    matmul's stationary has spare M), so h2's two q-groups pack the same way.
  - exp on ACT (scale=1/8), bf16 out; mask = DVE bf16 multiply (4x mode).
  - P@V with [V|1] stationary -> out^T plus denominator row, accumulated over
    windows in PSUM; normalization via K=1 broadcast matmul + DVE.
  - stage A for h2 is emitted AFTER stage B of h0/h1 so the Tile scheduler
    uses it as PE filler while ACT chews exp (keeps the PE HAM-warm).
  - O-projection: h0/h1 as one K=128 matmul, h2 K=64; output DMA'd as bf16.
"""

import sys

sys.path.insert(0, "/opt/trn_rl_repo")

import numpy as np
import ml_dtypes

BF16 = ml_dtypes.bfloat16

B = 2
S = 2048
D = 768
DK = 64
WIN = 256
NW = S // WIN   # 8 windows
NHC = 3         # heads per core
DH = NHC * DK   # 192
NT = S // 128   # 16 q tiles

_CACHE = {}


def _build_program():
    import concourse.tile as tile
    from concourse import mybir, bacc
    from contextlib import ExitStack

    f32 = mybir.dt.float32
    f32r = mybir.dt.float32r
    bf16 = mybir.dt.bfloat16
    Exp = mybir.ActivationFunctionType.Exp
    mult = mybir.AluOpType.mult

    nc = bacc.Bacc("TRN2", target_bir_lowering=False, debug=False)

    # xB: [128, 12288] packed as (n 4, k 6, s 512) so each of the 4 DMAs moves
    # 6KB-per-partition lines; wP: [128, 4224] = per k-chunk [w1 256|w2 256|wv 192]
    xB = nc.dram_tensor("xB", [128, 12288], bf16, kind="ExternalInput").ap()
    wP = nc.dram_tensor("wP", [128, 4224], bf16, kind="ExternalInput").ap()
    wo1 = nc.dram_tensor("wo1", [128, D], bf16, kind="ExternalInput").ap()
    wo2 = nc.dram_tensor("wo2", [64, D], bf16, kind="ExternalInput").ap()
    btA = nc.dram_tensor("btA", [128, 1], f32, kind="ExternalInput").ap()
    btB = nc.dram_tensor("btB", [128, 1], f32, kind="ExternalInput").ap()
    btC = nc.dram_tensor("btC", [128, 1], f32, kind="ExternalInput").ap()
    btD = nc.dram_tensor("btD", [128, 1], f32, kind="ExternalInput").ap()
    bvb = nc.dram_tensor("bvb", [128, 192], f32, kind="ExternalInput").ap()
    triu = nc.dram_tensor("triu", [128, 128], bf16, kind="ExternalInput").ap()
    onesb = nc.dram_tensor("onesb", [128, 64], bf16, kind="ExternalInput").ap()
    onesr = nc.dram_tensor("onesr", [1, 64], f32r, kind="ExternalInput").ap()
    out = nc.dram_tensor("out", [S, D], bf16, kind="ExternalOutput").ap()

    with tile.TileContext(nc) as tc, ExitStack() as ctx:
        consts = ctx.enter_context(tc.tile_pool(name="consts", bufs=1))
        qkv = ctx.enter_context(tc.tile_pool(name="qkv", bufs=1))

        xtp = ctx.enter_context(tc.tile_pool(name="xtp", bufs=1))
        xB_sb = xtp.tile([128, 12288], bf16, tag="xB")
        wP_sb = consts.tile([128, 4224], bf16, tag="wP")
        nc.sync.dma_start(out=wP_sb, in_=wP)
        for n in range(4):
            nc.sync.dma_start(out=xB_sb[:, 3072 * n:3072 * (n + 1)],
                              in_=xB[:, 3072 * n:3072 * (n + 1)])

        def xn_sl(k, n):
            return xB_sb[:, 3072 * n + 512 * k:3072 * n + 512 * k + 512]

        def xst_sl(k, st):
            c = 3072 * (st // 4) + 512 * k + 128 * (st % 4)
            return xB_sb[:, c:c + 128]

        w1_sb = [wP_sb[:, 704 * k:704 * k + 256] for k in range(6)]
        w2_sb = [wP_sb[:, 704 * k + 256:704 * k + 512] for k in range(6)]
        wv_sb = [wP_sb[:, 704 * k + 512:704 * k + 704] for k in range(6)]
        wo1_sb = consts.tile([128, D], bf16, tag="wo1")
        wo2_sb = consts.tile([64, D], bf16, tag="wo2")
        nc.sync.dma_start(out=wo1_sb, in_=wo1)
        nc.sync.dma_start(out=wo2_sb, in_=wo2)
        btA_sb = consts.tile([128, 1], f32, tag="btA")
        btB_sb = consts.tile([128, 1], f32, tag="btB")
        btC_sb = consts.tile([128, 1], f32, tag="btC")
        btD_sb = consts.tile([128, 1], f32, tag="btD")
        nc.sync.dma_start(out=btA_sb, in_=btA)
        nc.sync.dma_start(out=btB_sb, in_=btB)
        nc.sync.dma_start(out=btC_sb, in_=btC)
        nc.sync.dma_start(out=btD_sb, in_=btD)
        bvb_sb = consts.tile([128, 192], f32, tag="bvb")
        nc.sync.dma_start(out=bvb_sb, in_=bvb)
        triu_sb = consts.tile([128, 128], bf16, tag="triu")
        nc.sync.dma_start(out=triu_sb, in_=triu)
        onesb_sb = consts.tile([128, 64], bf16, tag="onesb")
        nc.sync.dma_start(out=onesb_sb, in_=onesb)
        onesr_sb = consts.tile([1, 64], f32r, tag="onesr")
        nc.sync.dma_start(out=onesr_sb, in_=onesr)

        # ---- long-lived activation tiles (bf16) ----
        qAB = qkv.tile([128, S], bf16, tag="qAB")  # [qT_h0|qT_h1], q-permuted
        kAB = qkv.tile([128, S], bf16, tag="kAB")  # [kT_h0|kT_h1], natural
        qC2 = qkv.tile([128, S], bf16, tag="qC2")  # qT_h2 dup'd, permuted
        kC2 = qkv.tile([128, S], bf16, tag="kC2")  # kT_h2 dup'd, natural
        v_sb = [qkv.tile([128, 195], bf16, tag=f"v{i}", name=f"vsb{i}")
                for i in range(NT)]
        attn01 = qkv.tile([128, S], bf16, tag="attn01")  # h0 parts 0-63, h1 64-127
        attn2 = qkv.tile([64, S], bf16, tag="attn2")

        def permuted_copy(dst, ps, n, bias):
            """psum 512-span n -> dst cols with even/odd tile permutation."""
            pr3 = ps.rearrange("p (c two k) -> p c two k", two=2, k=128)
            nc.vector.tensor_scalar_add(
                out=dst[:, 256 * n:256 * n + 256].rearrange("p (c k) -> p c k", k=128),
                in0=pr3[:, :, 0, :], scalar1=bias)
            nc.vector.tensor_scalar_add(
                out=dst[:, 1024 + 256 * n:1024 + 256 * n + 256].rearrange(
                    "p (c k) -> p c k", k=128),
                in0=pr3[:, :, 1, :], scalar1=bias)

        # ---- stage A for h0/h1 + V for all heads ----
        with tc.tile_pool(name="psA", bufs=2, space="PSUM") as psA:
            for n in range(4):
                xn = [xn_sl(k, n) for k in range(6)]
                psa = psA.tile([128, 512], f32, tag="psA")
                for k in range(6):
                    nc.tensor.matmul(psa, w1_sb[k][:, 0:128], xn[k],
                                     start=(k == 0), stop=(k == 5))
                permuted_copy(qAB, psa, n, btA_sb)
                psb = psA.tile([128, 512], f32, tag="psA")
                for k in range(6):
                    nc.tensor.matmul(psb, w1_sb[k][:, 128:256], xn[k],
                                     start=(k == 0), stop=(k == 5))
                nc.vector.tensor_scalar_add(
                    out=kAB[:, 512 * n:512 * (n + 1)], in0=psb, scalar1=btB_sb)

        # ---- stage B pools (+ filler projection pool for h2) ----
        triu_b = triu_sb.unsqueeze(1)

        with tc.tile_pool(name="sc", bufs=2, space="PSUM") as scp, \
             tc.tile_pool(name="po", bufs=3, space="PSUM") as pop, \
             tc.tile_pool(name="aps", bufs=1, space="PSUM") as aps, \
             tc.tile_pool(name="pt", bufs=8) as ptp, \
             tc.tile_pool(name="nrm", bufs=4) as nrm:

            def v_proj(st):
                """V projection for one 128-seq tile (filler-pool PSUM)."""
                psv = aps.tile([128, 192], f32, tag="apsA")
                for k in range(6):
                    nc.tensor.matmul(
                        psv, xst_sl(k, st),
                        wv_sb[k], start=(k == 0), stop=(k == 5))
                vt = v_sb[st]
                nc.vector.tensor_tensor(
                    out=vt.rearrange("p (h c) -> p h c", c=65)[:, :, 0:64],
                    in0=psv.rearrange("p (h c) -> p h c", c=64),
                    in1=bvb_sb.rearrange("p (h c) -> p h c", c=64),
                    op=mybir.AluOpType.add)
                nc.vector.tensor_copy(
                    out=vt.rearrange("p (h c) -> p h c", c=65)[:, :, 64:65],
                    in_=onesb_sb[:, 0:3].unsqueeze(2))

            def norm(po, dst_rows, dst_cols):
                """po [65,512]: rows 0-63 = out^T, row 64 = denom.
                dst = attn tile slice [64, 512]."""
                den = nrm.tile([1, 512], f32r, tag="den")
                nc.vector.tensor_copy(out=den, in_=po[64:65, :])
                dps = pop.tile([64, 512], f32, tag="po")
                nc.tensor.matmul(dps, onesr_sb, den, start=True, stop=True)
                rec = nrm.tile([64, 512], f32, tag="rec")
                nc.vector.reciprocal_approx_fast(out=rec, in_=dps)
                nc.vector.tensor_tensor(
                    out=dst_rows[:, dst_cols], in0=po[0:64, :], in1=rec, op=mult)

            def exp_unit(sc, mask_lo, mask_hi):
                pt = ptp.tile([128, 1024], bf16, tag="pt")
                nc.scalar.activation(out=pt, in_=sc, func=Exp, scale=0.125)
                if mask_lo and mask_hi:
                    p3 = pt.rearrange("p (c k) -> p c k", k=128)
                    nc.vector.tensor_tensor(
                        out=p3, in0=p3, in1=triu_b.broadcast_to([128, 8, 128]),
                        op=mult)
                elif mask_lo or mask_hi:
                    off = 0 if mask_lo else 512
                    p3 = pt[:, off:off + 512].rearrange("p (c k) -> p c k", k=128)
                    nc.vector.tensor_tensor(
                        out=p3, in0=p3, in1=triu_b.broadcast_to([128, 4, 128]),
                        op=mult)
                return pt

            # ---- B01: heads h0,h1 packed on partition halves ----
            def b01(g, hh, pre=None):
                qc = slice(1024 * g + 512 * hh, 1024 * g + 512 * hh + 512)
                po0 = pop.tile([65, 512], f32, tag="po")
                po1 = pop.tile([65, 512], f32, tag="po")
                for w in range(NW):
                    if pre is not None:
                        pre(w)
                    klo = slice(WIN * w, WIN * w + 128)
                    sc = scp.tile([128, 1024], f32, tag="sc")
                    nc.tensor.matmul(sc[:, 0:512], kAB[0:64, klo],
                                     qAB[0:64, qc], start=True, stop=True)
                    nc.tensor.matmul(sc[:, 512:1024], kAB[64:128, klo],
                                     qAB[64:128, qc], start=True, stop=True)
                    pt = exp_unit(sc, mask_lo=(g == 0), mask_hi=(g == 0))
                    last = (g == 0 and w == NW - 1)
                    nc.tensor.matmul(po0, v_sb[2 * w][:, 0:65],
                                     pt[:, 0:512], start=(w == 0), stop=last)
                    nc.tensor.matmul(po1, v_sb[2 * w][:, 65:130],
                                     pt[:, 512:1024], start=(w == 0), stop=last)
                    if g == 1:
                        khi = slice(WIN * w + 128, WIN * w + 256)
                        sch = scp.tile([128, 1024], f32, tag="sc")
                        nc.tensor.matmul(sch[:, 0:512], kAB[0:64, khi],
                                         qAB[0:64, qc], start=True, stop=True)
                        nc.tensor.matmul(sch[:, 512:1024], kAB[64:128, khi],
                                         qAB[64:128, qc], start=True, stop=True)
                        pth = exp_unit(sch, mask_lo=True, mask_hi=True)
                        last = (w == NW - 1)
                        nc.tensor.matmul(po0, v_sb[2 * w + 1][:, 0:65],
                                         pth[:, 0:512], start=False, stop=last)
                        nc.tensor.matmul(po1, v_sb[2 * w + 1][:, 65:130],
                                         pth[:, 512:1024], start=False, stop=last)
                cols = slice(1024 * g + 512 * hh, 1024 * g + 512 * hh + 512)
                norm(po0, attn01[0:64, :], cols)
                norm(po1, attn01[64:128, :], cols)

            def first_pre(w):
                v_proj(2 * w)
                v_proj(2 * w + 1)

            b01(0, 0, pre=first_pre)
            b01(0, 1)
            b01(1, 0)
            b01(1, 1)

            # ---- filler: stage A for h2 (emitted after B01 -> lower priority,
            # fills PE idle while ACT works through B01's exps) ----
            for n in range(4):
                xn = [xn_sl(k, n) for k in range(6)]
                psq = aps.tile([128, 512], f32, tag="apsA")
                for k in range(6):
                    nc.tensor.matmul(psq, w2_sb[k][:, 0:128], xn[k],
                                     start=(k == 0), stop=(k == 5))
                permuted_copy(qC2, psq, n, btC_sb)
            for n in range(4):
                xn = [xn_sl(k, n) for k in range(6)]
                psk = aps.tile([128, 512], f32, tag="apsA")
                for k in range(6):
                    nc.tensor.matmul(psk, w2_sb[k][:, 128:256], xn[k],
                                     start=(k == 0), stop=(k == 5))
                nc.vector.tensor_scalar_add(
                    out=kC2[:, 512 * n:512 * (n + 1)], in0=psk, scalar1=btD_sb)

            # ---- B2: head h2, groups g0/g1 packed on partition halves ----
            def b2(hh):
                q0 = slice(512 * hh, 512 * hh + 512)            # grp0 cols
                q1 = slice(1024 + 512 * hh, 1024 + 512 * hh + 512)  # grp1 cols
                pg0 = pop.tile([65, 512], f32, tag="po")
                pg1 = pop.tile([65, 512], f32, tag="po")
                for w in range(NW):
                    klo = slice(WIN * w, WIN * w + 128)
                    khi = slice(WIN * w + 128, WIN * w + 256)
                    sc = scp.tile([128, 1024], f32, tag="sc")
                    nc.tensor.matmul(sc[:, 0:512], kC2[0:64, klo],
                                     qC2[0:64, q0], start=True, stop=True)
                    nc.tensor.matmul(sc[:, 512:1024], kC2[64:128, klo],
                                     qC2[64:128, q1], start=True, stop=True)
                    pt = exp_unit(sc, mask_lo=True, mask_hi=False)
                    nc.tensor.matmul(pg0, v_sb[2 * w][:, 130:195],
                                     pt[:, 0:512], start=(w == 0),
                                     stop=(w == NW - 1))
                    nc.tensor.matmul(pg1, v_sb[2 * w][:, 130:195],
                                     pt[:, 512:1024], start=(w == 0), stop=False)
                    scb = scp.tile([128, 512], f32, tag="sc")
                    nc.tensor.matmul(scb, kC2[0:64, khi], qC2[0:64, q1],
                                     start=True, stop=True)
                    ptb = ptp.tile([128, 512], bf16, tag="pt")
                    nc.scalar.activation(out=ptb, in_=scb, func=Exp, scale=0.125)
                    p3 = ptb.rearrange("p (c k) -> p c k", k=128)
                    nc.vector.tensor_tensor(
                        out=p3, in0=p3, in1=triu_b.broadcast_to([128, 4, 128]),
                        op=mult)
                    nc.tensor.matmul(pg1, v_sb[2 * w + 1][:, 130:195],
                                     ptb, start=False, stop=(w == NW - 1))
                norm(pg0, attn2, slice(512 * hh, 512 * hh + 512))
                norm(pg1, attn2, slice(1024 + 512 * hh, 1024 + 512 * hh + 512))

            for hh in range(2):
                b2(hh)

        # ---- stage C ----
        with tc.tile_pool(name="oc", bufs=3, space="PSUM") as oc_pool, \
             tc.tile_pool(name="ost", bufs=3) as ost_pool:
            for p in range(NT):
                pso = oc_pool.tile([128, D], f32, tag="pso")
                pcols = slice(128 * p, 128 * (p + 1))
                for (n0, n1) in ((0, 512), (512, 768)):
                    nc.tensor.matmul(pso[:, n0:n1], attn01[:, pcols],
                                     wo1_sb[:, n0:n1], start=True, stop=False)
                    nc.tensor.matmul(pso[:, n0:n1], attn2[:, pcols],
                                     wo2_sb[:, n0:n1], start=False, stop=True)
                ot = ost_pool.tile([128, D], bf16, tag="ot")
                if p % 2 == 0:
                    nc.scalar.copy(out=ot, in_=pso)
                else:
                    nc.vector.tensor_copy(out=ot, in_=pso)
                t = 2 * p if p < 8 else 2 * (p - 8) + 1
                nc.sync.dma_start(out=out[128 * t:128 * (t + 1), :], in_=ot)

    nc.compile()
    return nc


def _prep_core_inputs(inputs, c):
    x = inputs["x"]
    Wq, bq = inputs["Wq"], inputs["bq"]
    Wk, bk = inputs["Wk"], inputs["bk"]
    Wv, bv = inputs["Wv"], inputs["bv"]
    Wo = inputs["Wo"]
    b = c // 4
    r0 = (c % 4) * DH  # first feature row of this core's 192-row head block

    xT = np.asarray(x[b]).T.astype(np.float32)  # [768, 2048]
    W1 = np.concatenate(
        [Wq[r0:r0 + 128].T, Wk[r0:r0 + 128].T], axis=1)
    q2 = Wq[r0 + 128:r0 + 192].T
    k2 = Wk[r0 + 128:r0 + 192].T
    W2 = np.concatenate([q2, q2, k2, k2], axis=1)
    Wvp = Wv[r0:r0 + 192].T
    # packed x: [128, (n 4, k 6, s 512)]
    xBp = np.zeros((128, 12288), np.float32)
    for n in range(4):
        for k in range(6):
            xBp[:, 3072 * n + 512 * k:3072 * n + 512 * (k + 1)] = \
                xT[128 * k:128 * (k + 1), 512 * n:512 * (n + 1)]
    # packed weights: [128, (k 6, [w1 256 | w2 256 | wv 192])]
    wPp = np.zeros((128, 4224), np.float32)
    for k in range(6):
        wPp[:, 704 * k:704 * k + 256] = W1[128 * k:128 * (k + 1)]
        wPp[:, 704 * k + 256:704 * k + 512] = W2[128 * k:128 * (k + 1)]
        wPp[:, 704 * k + 512:704 * k + 704] = Wvp[128 * k:128 * (k + 1)]
    bq2 = bq[r0 + 128:r0 + 192]
    bk2 = bk[r0 + 128:r0 + 192]
    bvb = np.zeros((128, 192), np.float32)
    for h in range(NHC):
        bvb[:, 64 * h:64 * h + 64] = bv[r0 + 64 * h:r0 + 64 * (h + 1)][None, :]

    return dict(
        xB=xBp.astype(BF16), wP=wPp.astype(BF16),
        wo1=np.ascontiguousarray(Wo[:, r0:r0 + 128].T).astype(BF16),
        wo2=np.ascontiguousarray(Wo[:, r0 + 128:r0 + 192].T).astype(BF16),
        btA=np.ascontiguousarray(bq[r0:r0 + 128].reshape(128, 1)).astype(np.float32),
        btB=np.ascontiguousarray(bk[r0:r0 + 128].reshape(128, 1)).astype(np.float32),
        btC=np.concatenate([bq2, bq2]).reshape(128, 1).astype(np.float32),
        btD=np.concatenate([bk2, bk2]).reshape(128, 1).astype(np.float32),
        bvb=np.ascontiguousarray(bvb),
        triu=np.triu(np.ones((128, 128), np.float32)).astype(BF16),
        onesb=np.ones((128, 64), np.float32).astype(BF16),
        onesr=np.ones((1, 64), np.float32),
    )


def _install_ntff_hook():
    """Register antenv.axon_hooks with a ctypes NTFF profile hook so
    run_bass_kernel_spmd(trace=True) can capture device-side exec time."""
    import types, ctypes, contextlib

    try:
        import antenv.axon_hooks  # noqa: F401
        return
    except ImportError:
        pass
    so_path = "/opt/axon/libaxon_pjrt.so"
    lib = ctypes.CDLL(so_path)
    if not hasattr(lib, "axon_start_nrt_profile"):
        return
    lib.axon_start_nrt_profile.argtypes = [
        ctypes.POINTER(ctypes.c_int64), ctypes.c_size_t]
    lib.axon_start_nrt_profile.restype = ctypes.c_int64
    lib.axon_stop_nrt_profile.argtypes = [ctypes.c_char_p]
    lib.axon_stop_nrt_profile.restype = ctypes.c_int64

    @contextlib.contextmanager
    def _hook(output_dir, device_ids):
        import jax
        jax.devices()
        if device_ids:
            ids = (ctypes.c_int64 * len(device_ids))(*device_ids)
            rc = lib.axon_start_nrt_profile(ids, len(device_ids))
        else:
            rc = lib.axon_start_nrt_profile(None, 0)
        if rc != 0:
            raise RuntimeError(f"axon_start_nrt_profile rc={rc}")
        try:
            yield
        finally:
            n = lib.axon_stop_nrt_profile(str(output_dir).encode())
            print(f"profile: {n} file(s) written to {output_dir}")

    mod = types.ModuleType("antenv.axon_hooks")
    mod.get_axon_ntff_profile_hook = lambda: _hook
    mod.set_axon_ntff_profile_hook = lambda h: None
    sys.modules["antenv.axon_hooks"] = mod
    import antenv
    antenv.axon_hooks = mod


def kernel(**inputs):
    import os
    from concourse import bass_utils

    if "nc" not in _CACHE:
        _CACHE["nc"] = _build_program()
    nc = _CACHE["nc"]

    trace = bool(os.environ.get("MHA_TRACE"))
    kwargs = {}
    if trace:
        _install_ntff_hook()
        kwargs = dict(trace=True, tmpdir="/tmp/mha_trace")
        os.makedirs("/tmp/mha_trace", exist_ok=True)

    in_maps = [_prep_core_inputs(inputs, c) for c in range(8)]
    res = bass_utils.run_bass_kernel_spmd(
        nc, in_maps, core_ids=list(range(8)), **kwargs)
    _CACHE["last_results"] = res
    if trace and res.exec_time_ns is not None:
        print(f"HW exec time: {res.exec_time_ns} ns")
    out = np.zeros((B, S, D), np.float32)
    for c in range(8):
        out[c // 4] += res.results[c]["out"].astype(np.float32)
    out += np.asarray(inputs["bo"], np.float32).reshape(1, 1, D)
    return out
